# revision 44
# baseline (speedup 1.0000x reference)
"""Block-sparse self-attention (DeepSpeed "fixed" layout) on 8 trn2 cores.

Problem: B=2, H=16, S=2048, D=64 fp32. Mask (identical for every head,
numverts=1): each 64-wide diagonal window is dense, plus every 4th
16-col block ("stripe") is attended by all queries. Per 64-row query
window the attended key set = 512 stripe cols + 48 non-stripe window
cols.

Sharding: 32 (b,h) pairs -> 4 per core (batch+head parallel).

Host prep per pair (pure layout + dtype cast; reorder puts the 512
stripe cols first, then 32 windows x 48 non-stripe cols):
  qT  [64, 2048]  Q^T
  kT  [64, 2560]  K^T reordered: 512 stripe cols, then 16 window-pairs
                  of 128 cols each [48 even | 16 zero | 48 odd | 16 zero]
                  (zero padding keeps engine partition bases 32-aligned)
  vs  [128, 4*65] stripe V_aug in on-chip layout: partition r, chunk c
                  holds V[reorder[c*128+r]] ++ [1]  (ones col -> softmax
                  denominator L rides the PV matmul)
  vw2 [112, 16*65] window-pair V_aug: partitions 0:48 = window 2i,
                  64:112 = window 2i+1 (48:64 zero), ones col each

On chip per pair (fp16 operands, software-pipelined across pairs so the
tensor engine never idles: pair p's QK fills interleave with pair p-1's
PV accumulation, drains split across ACT and DVE):
  stripe scores  S^T[k,q] = matmul(kt chunk, qt)      16x [128,512]
  window scores  [112,128] blocks: windows (2j, 2j+1) stacked on the
                 output-partition axis share one matmul; the off-
                 diagonal cross-window entries are computed but thrown
                 away by the block-diagonal exp-write into a zeroed P
  P = exp(0.125 * S): ACT engine for 7/8 stripe tiles (exact), DVE
                 Schraudolph-in-fp16-bits for the rest (~30% of P,
                 ~1.5% rms elementwise) to halve the ACT pole
  O'^T[65,q] accumulates stripe chunks ([65,512] x16) and window pairs
                 ([65,128] x16); row 64 = L (ones columns of V_aug)
  out[p] = O' [65, 2048] as fp16 via DVE copy + 2 DMAs
Host: O = (O'[0:64] / O'[64])^T per pair.
"""

import numpy as np

B, H, S, D = 2, 16, 2048, 64
NPAIRS = B * H
NCORES = 8
P_PER_CORE = NPAIRS // NCORES  # 4
NCH = 4        # stripe k-chunks of 128
NW = S // 64   # 32 windows
SCALE = float(D) ** -0.5


def _reorder_idx():
    blocks = np.arange(S // 16)
    stripe = blocks[blocks % 4 == 3]
    rest = blocks[blocks % 4 != 3]
    cols = np.arange(S).reshape(-1, 16)
    return np.concatenate([cols[stripe].ravel(), cols[rest].ravel()])


_REORDER = _reorder_idx()

_CACHE = {}


def _build(dt_in_name="float16", npairs=P_PER_CORE):
    from contextlib import ExitStack
    import concourse.bacc as bacc
    import concourse.tile as tile
    from concourse import mybir

    dt_in = getattr(mybir.dt, dt_in_name)
    f32 = mybir.dt.float32
    i16 = mybir.dt.int16
    EXP = mybir.ActivationFunctionType.Exp
    MUL = mybir.AluOpType.mult
    ADD = mybir.AluOpType.add
    # Schraudolph exp in fp16 bit space: fp16_bits(exp(s*SCALE)) ~
    # s * (SCALE*1024*log2 e) + (15*1024 - 59.3). One DVE tensor_scalar
    # (fp32 PSUM -> int16 convert) per tile; the int16 buffer is the fp16
    # P tile by bitcast. ~1.5% rms elementwise, applied to ~30% of P.
    SCH_A = SCALE * 1024.0 / float(np.log(2.0))
    SCH_B = 15.0 * 1024.0 - 59.3

    nc = bacc.Bacc("TRN2", target_bir_lowering=False, debug=False,
                   num_devices=NCORES)
    KW = 512 + 64 * NW  # padded kT width (2560)
    VW = NCH * 65 + (NW // 2) * 65  # 260 + 1040
    # layout: [kt_stripe 512 | qt 2048 | kt_windows 2048]
    qkT = nc.dram_tensor("qkT", [npairs, 64, S + KW], dt_in,
                         kind="ExternalInput").ap()
    vin = nc.dram_tensor("vin", [npairs, 128, VW], dt_in,
                         kind="ExternalInput").ap()
    out = nc.dram_tensor("out", [npairs, 65, S], dt_in,
                         kind="ExternalOutput").ap()

    with tile.TileContext(nc) as tc, ExitStack() as ctx:
        qk_pool = ctx.enter_context(tc.tile_pool(name="qk", bufs=2))
        v_pool = ctx.enter_context(tc.tile_pool(name="v", bufs=2))
        p_pool = ctx.enter_context(tc.tile_pool(name="p", bufs=2))
        s_pool = ctx.enter_context(tc.tile_pool(name="s", bufs=3, space="PSUM"))
        o_pool = ctx.enter_context(tc.tile_pool(name="o", bufs=2, space="PSUM"))

        # P-window tiles are persistent: the zero cross-window blocks are
        # zeroed once and stay zero (every pair overwrites only the same
        # diagonal blocks), so no per-pair memset is needed.
        pw_tiles = [p_pool.tile([112, S], dt_in, tag=f"pw{b}",
                                name=f"pw_{b}") for b in range(2)]

        def load_tiles(p):
            qk = qk_pool.tile([64, S + KW], dt_in, tag="qk")
            # stripe-K + first q-half first (all that rounds 0-3 need),
            # the rest behind it on a second queue. For pair 0 the first
            # block is split across two queues (Scalar is idle then) to
            # pull the very first fill earlier.
            if p == 0:
                nc.sync.dma_start(out=qk[:, 0:768], in_=qkT[p, :, 0:768])
                nc.scalar.dma_start(out=qk[:, 768:1536],
                                    in_=qkT[p, :, 768:1536])
            else:
                nc.sync.dma_start(out=qk[:, 0:1536],
                                  in_=qkT[p, :, 0:1536])
            nc.gpsimd.dma_start(out=qk[:, 1536:2560],
                                in_=qkT[p, :, 1536:2560])
            nc.gpsimd.dma_start(out=qk[:, 2560:4608],
                                in_=qkT[p, :, 2560:4608])
            vt = v_pool.tile([128, VW], dt_in, tag="v")
            nc.sync.dma_start(out=vt, in_=vin[p])
            ps = p_pool.tile([128, NCH, S], dt_in, tag="ps")
            return dict(p=p, kt=qk[:, 0:512], qt=qk[:, 512:2560],
                        ktw=qk[:, 2560:4608],
                        vst=vt[:, 0:NCH * 65],
                        vwt=vt[0:112, NCH * 65:VW], ps=ps,
                        pw=pw_tiles[p % 2])

        def pv_step(cx, i):
            # i in 0..31: per q-quarter qg: 4 stripe MMs then 4 window MMs.
            # O'^T accumulates in a [65, 512] quarter; V_aug ones col lands
            # the softmax denominator L in row 64.
            qg, r = i // 8, i % 8
            if r == 0:
                cx["ov" + str(qg)] = o_pool.tile([128, 512], f32, tag="o",
                                                 name=f"ov{cx['p']}_{qg}")
            ov = cx["ov" + str(qg)]
            if r < 4:
                c = r
                nc.tensor.matmul(
                    out=ov[0:65, :],
                    lhsT=cx["vst"][:, c * 65:(c + 1) * 65],
                    rhs=cx["ps"][:, c, qg * 512:(qg + 1) * 512],
                    start=(c == 0), stop=False, skip_group_check=True)
            else:
                j = 4 * qg + (r - 4)
                q0 = (j % 4) * 128
                nc.tensor.matmul(
                    out=ov[0:65, q0:q0 + 128],
                    lhsT=cx["vwt"][:, j * 65:(j + 1) * 65],
                    rhs=cx["pw"][0:112, 128 * j:128 * j + 128],
                    start=False, stop=(r == 7), skip_group_check=True)

        def pv_copy(cx, qg):
            # PSUM -> SBUF staging (DMA cannot read PSUM), all on DVE,
            # fp32 -> fp16 in the copy; out-DMA per half on two queues
            if qg == 0:
                cx["ob"] = p_pool.tile([65, S], dt_in, tag="ob",
                                       name=f"ob{cx['p']}")
            ob = cx["ob"]
            ov = cx["ov" + str(qg)]
            nc.vector.tensor_copy(ob[:, qg * 512:(qg + 1) * 512], ov[0:65, :])
            if qg == 1:
                nc.sync.dma_start(out=out[cx["p"], :, 0:1024],
                                  in_=ob[:, 0:1024])
            if qg == 3:
                nc.gpsimd.dma_start(out=out[cx["p"], :, 1024:2048],
                                    in_=ob[:, 1024:2048])

        ctxs = [load_tiles(0)]
        for pwb in pw_tiles:
            nc.gpsimd.memset(pwb, 0.0)
        for p in range(npairs):
            nxt_needed = p + 1 < npairs
            cur = ctxs[p]
            prev = ctxs[p - 1] if p > 0 else None
            # emit, prefetching next pair's tiles after the first round
            qt, kt, ps, pw = cur["qt"], cur["kt"], cur["ps"], cur["pw"]
            for r in range(8):
                c, g = r % 4, r // 4
                st = s_pool.tile([128, 1024], f32, tag="s")
                for u in range(2):
                    q0 = g * 1024 + u * 512
                    nc.tensor.matmul(
                        out=st[:, u * 512:(u + 1) * 512],
                        lhsT=kt[:, c * 128:(c + 1) * 128],
                        rhs=qt[:, q0:q0 + 512],
                        start=True, stop=True)
                po = ps[:, c, g * 1024:(g + 1) * 1024]
                if c == 3:
                    nc.vector.tensor_scalar(
                        out=po.bitcast(i16), in0=st,
                        scalar1=SCH_A, scalar2=SCH_B, op0=MUL, op1=ADD)
                else:
                    nc.scalar.activation(out=po, in_=st,
                                         func=EXP, scale=SCALE)
                if r == 0 and nxt_needed:
                    ctxs.append(load_tiles(p + 1))
                if prev is not None:
                    for i in range(4 * r, 4 * r + 4):
                        pv_step(prev, i)
                    if r % 2 == 1:
                        pv_copy(prev, r // 2)
            for h in range(2):
                sw = s_pool.tile([128, 1024], f32, tag="s")
                for j in range(8 * h, 8 * h + 8):
                    fo = (j - 8 * h) * 128
                    nc.tensor.matmul(
                        out=sw[0:112, fo:fo + 128],
                        lhsT=cur["ktw"][:, 128 * j:128 * j + 112],
                        rhs=qt[:, 128 * j:128 * j + 128],
                        start=True, stop=True)
                sw4 = sw.rearrange("p (j t f) -> p j t f", t=2, f=64)
                pw4 = pw.rearrange("p (j t f) -> p j t f", t=2, f=64)
                nc.vector.tensor_scalar(
                    out=pw4[0:48, 8 * h:8 * h + 8, 0, :].bitcast(i16),
                    in0=sw4[0:48, 0:8, 0, :],
                    scalar1=SCH_A, scalar2=SCH_B, op0=MUL, op1=ADD)
                nc.vector.tensor_scalar(
                    out=pw4[64:112, 8 * h:8 * h + 8, 1, :].bitcast(i16),
                    in0=sw4[64:112, 0:8, 1, :],
                    scalar1=SCH_A, scalar2=SCH_B, op0=MUL, op1=ADD)
        # flush: PV of the last pair
        last = ctxs[-1]
        for i in range(32):
            pv_step(last, i)
            if i % 8 == 7:
                pv_copy(last, i // 8)

    nc.compile()
    return nc


def _get_nc(dt_in_name="float16"):
    if dt_in_name not in _CACHE:
        _CACHE[dt_in_name] = _build(dt_in_name)
    return _CACHE[dt_in_name]


def _prep_inputs(query, key, value, np_dt):
    q = np.asarray(query).reshape(NPAIRS, S, D)
    k = np.asarray(key).reshape(NPAIRS, S, D)
    v = np.asarray(value).reshape(NPAIRS, S, D)
    kr = k[:, _REORDER, :]
    vr = v[:, _REORDER, :]
    # qkT: [stripe K^T 512 | Q^T 2048 | window K^T padded: pairs of
    # 128 cols as [48 even | 16 zero | 48 odd | 16 zero]]
    KW = 512 + 64 * NW
    qkT = np.zeros((NPAIRS, 64, S + KW), np_dt)
    kTs = kr.transpose(0, 2, 1).astype(np_dt)  # [P, 64, 2048]
    qkT[:, :, 0:512] = kTs[:, :, 0:512]
    qkT[:, :, 512:512 + S] = q.transpose(0, 2, 1)
    kw = kTs[:, :, 512:].reshape(NPAIRS, 64, NW // 2, 2, 48)
    kTw = qkT[:, :, 512 + S:].reshape(NPAIRS, 64, NW // 2, 2, 64)
    kTw[:, :, :, :, 0:48] = kw
    va = np.concatenate(
        [vr, np.ones((NPAIRS, S, 1), vr.dtype)], axis=2).astype(np_dt)
    # vin: stripe V_aug [partition r, chunk c, 65] ++ window-pair V_aug
    # [112 rows (0:48 window 2i, 64:112 window 2i+1, 48:64 zero), i, 65]
    VW = NCH * 65 + (NW // 2) * 65
    vin = np.zeros((NPAIRS, 128, VW), np_dt)
    vin[:, :, 0:NCH * 65] = (
        va[:, :512].reshape(NPAIRS, NCH, 128, 65).transpose(0, 2, 1, 3)
    ).reshape(NPAIRS, 128, NCH * 65)
    vw = va[:, 512:].reshape(NPAIRS, NW // 2, 2, 48, 65)
    vwin = vin[:, :, NCH * 65:].reshape(NPAIRS, 128, NW // 2, 65)
    vwin[:, 0:48] = vw[:, :, 0].transpose(0, 2, 1, 3)
    vwin[:, 64:112] = vw[:, :, 1].transpose(0, 2, 1, 3)
    in_maps = []
    for core in range(NCORES):
        sl = slice(core * P_PER_CORE, (core + 1) * P_PER_CORE)
        in_maps.append({"qkT": np.ascontiguousarray(qkT[sl]),
                        "vin": np.ascontiguousarray(vin[sl])})
    return in_maps


def _run(query, key, value, dt_in_name="float16", trace=False):
    from concourse.bass_utils import run_bass_kernel_spmd
    nc = _get_nc(dt_in_name)
    in_maps = _prep_inputs(query, key, value, np.float16
                           if dt_in_name == "float16" else np.float32)
    res = run_bass_kernel_spmd(nc, in_maps, list(range(NCORES)), trace=trace)
    o = np.concatenate([res.results[i]["out"] for i in range(NCORES)],
                       axis=0).astype(np.float32)
    full = (o[:, 0:64, :] / o[:, 64:65, :]).transpose(0, 2, 1).reshape(
        B, H, S, D).astype(np.float32)
    return full, res


def kernel(query, key, value):
    full, _ = _run(np.asarray(query), np.asarray(key), np.asarray(value))
    return full


# revision 45
# speedup vs baseline: 1.0107x; 1.0107x over previous
"""Block-sparse self-attention (DeepSpeed "fixed" layout) on 8 trn2 cores.

Problem: B=2, H=16, S=2048, D=64 fp32. Mask (identical for every head,
numverts=1): each 64-wide diagonal window is dense, plus every 4th
16-col block ("stripe") is attended by all queries. Per 64-row query
window the attended key set = 512 stripe cols + 48 non-stripe window
cols.

Sharding: 32 (b,h) pairs -> 4 per core (batch+head parallel).

Host prep per pair (pure layout + dtype cast; reorder puts the 512
stripe cols first, then 32 windows x 48 non-stripe cols):
  qT  [64, 2048]  Q^T
  kT  [64, 2560]  K^T reordered: 512 stripe cols, then 16 window-pairs
                  of 128 cols each [48 even | 16 zero | 48 odd | 16 zero]
                  (zero padding keeps engine partition bases 32-aligned)
  vs  [128, 4*65] stripe V_aug in on-chip layout: partition r, chunk c
                  holds V[reorder[c*128+r]] ++ [1]  (ones col -> softmax
                  denominator L rides the PV matmul)
  vw2 [112, 16*65] window-pair V_aug: partitions 0:48 = window 2i,
                  64:112 = window 2i+1 (48:64 zero), ones col each

On chip per pair (fp16 operands, software-pipelined across pairs so the
tensor engine never idles: pair p's QK fills interleave with pair p-1's
PV accumulation, drains split across ACT and DVE):
  stripe scores  S^T[k,q] = matmul(kt chunk, qt)      16x [128,512]
  window scores  [112,128] blocks: windows (2j, 2j+1) stacked on the
                 output-partition axis share one matmul; the off-
                 diagonal cross-window entries are computed but thrown
                 away by the block-diagonal exp-write into a zeroed P
  P = exp(0.125 * S): ACT engine for 6/8 stripe tiles (exact), DVE
                 Schraudolph-in-fp16-bits for chunk 3 + windows (~31%
                 of P, ~1.5% rms elementwise); keeps both drain engines
                 under the warm-clock PE pair period
  O'^T[65,q] accumulates stripe chunks ([65,512] x16) and window pairs
                 ([65,128] x16); row 64 = L (ones columns of V_aug)
  out[p] = O' [65, 2048] as fp16 via DVE copy + 2 DMAs
Host: O = (O'[0:64] / O'[64])^T per pair.
"""

import numpy as np

B, H, S, D = 2, 16, 2048, 64
NPAIRS = B * H
NCORES = 8
P_PER_CORE = NPAIRS // NCORES  # 4
NCH = 4        # stripe k-chunks of 128
NW = S // 64   # 32 windows
SCALE = float(D) ** -0.5


def _reorder_idx():
    blocks = np.arange(S // 16)
    stripe = blocks[blocks % 4 == 3]
    rest = blocks[blocks % 4 != 3]
    cols = np.arange(S).reshape(-1, 16)
    return np.concatenate([cols[stripe].ravel(), cols[rest].ravel()])


_REORDER = _reorder_idx()

_CACHE = {}


def _build(dt_in_name="float16", npairs=P_PER_CORE):
    from contextlib import ExitStack
    import concourse.bacc as bacc
    import concourse.tile as tile
    from concourse import mybir

    dt_in = getattr(mybir.dt, dt_in_name)
    f32 = mybir.dt.float32
    i16 = mybir.dt.int16
    EXP = mybir.ActivationFunctionType.Exp
    MUL = mybir.AluOpType.mult
    ADD = mybir.AluOpType.add
    # Schraudolph exp in fp16 bit space: fp16_bits(exp(s*SCALE)) ~
    # s * (SCALE*1024*log2 e) + (15*1024 - 59.3). One DVE tensor_scalar
    # (fp32 PSUM -> int16 convert) per tile; the int16 buffer is the fp16
    # P tile by bitcast. ~1.5% rms elementwise, applied to ~30% of P.
    SCH_A = SCALE * 1024.0 / float(np.log(2.0))
    SCH_B = 15.0 * 1024.0 - 59.3

    nc = bacc.Bacc("TRN2", target_bir_lowering=False, debug=False,
                   num_devices=NCORES)
    KW = 512 + 64 * NW  # padded kT width (2560)
    VW = NCH * 65 + (NW // 2) * 65  # 260 + 1040
    # layout: [kt_stripe 512 | qt 2048 | kt_windows 2048]
    qkT = nc.dram_tensor("qkT", [npairs, 64, S + KW], dt_in,
                         kind="ExternalInput").ap()
    vin = nc.dram_tensor("vin", [npairs, 128, VW], dt_in,
                         kind="ExternalInput").ap()
    out = nc.dram_tensor("out", [npairs, 65, S], dt_in,
                         kind="ExternalOutput").ap()

    with tile.TileContext(nc) as tc, ExitStack() as ctx:
        qk_pool = ctx.enter_context(tc.tile_pool(name="qk", bufs=2))
        v_pool = ctx.enter_context(tc.tile_pool(name="v", bufs=2))
        p_pool = ctx.enter_context(tc.tile_pool(name="p", bufs=2))
        s_pool = ctx.enter_context(tc.tile_pool(name="s", bufs=3, space="PSUM"))
        o_pool = ctx.enter_context(tc.tile_pool(name="o", bufs=2, space="PSUM"))

        # P-window tiles are persistent: the zero cross-window blocks are
        # zeroed once and stay zero (every pair overwrites only the same
        # diagonal blocks), so no per-pair memset is needed.
        pw_tiles = [p_pool.tile([112, S], dt_in, tag=f"pw{b}",
                                name=f"pw_{b}") for b in range(2)]

        def load_tiles(p):
            qk = qk_pool.tile([64, S + KW], dt_in, tag="qk")
            # stripe-K + first q-half first (all that rounds 0-3 need),
            # the rest behind it on a second queue. For pair 0 the first
            # block is split across two queues (Scalar is idle then) to
            # pull the very first fill earlier.
            if p == 0:
                nc.sync.dma_start(out=qk[:, 0:768], in_=qkT[p, :, 0:768])
                nc.scalar.dma_start(out=qk[:, 768:1536],
                                    in_=qkT[p, :, 768:1536])
            else:
                nc.sync.dma_start(out=qk[:, 0:1536],
                                  in_=qkT[p, :, 0:1536])
            nc.gpsimd.dma_start(out=qk[:, 1536:2560],
                                in_=qkT[p, :, 1536:2560])
            nc.gpsimd.dma_start(out=qk[:, 2560:4608],
                                in_=qkT[p, :, 2560:4608])
            vt = v_pool.tile([128, VW], dt_in, tag="v")
            nc.sync.dma_start(out=vt, in_=vin[p])
            ps = p_pool.tile([128, NCH, S], dt_in, tag="ps")
            return dict(p=p, kt=qk[:, 0:512], qt=qk[:, 512:2560],
                        ktw=qk[:, 2560:4608],
                        vst=vt[:, 0:NCH * 65],
                        vwt=vt[0:112, NCH * 65:VW], ps=ps,
                        pw=pw_tiles[p % 2])

        def pv_step(cx, i):
            # i in 0..31: per q-quarter qg: 4 stripe MMs then 4 window MMs.
            # O'^T accumulates in a [65, 512] quarter; V_aug ones col lands
            # the softmax denominator L in row 64.
            qg, r = i // 8, i % 8
            if r == 0:
                cx["ov" + str(qg)] = o_pool.tile([128, 512], f32, tag="o",
                                                 name=f"ov{cx['p']}_{qg}")
            ov = cx["ov" + str(qg)]
            if r < 4:
                c = r
                nc.tensor.matmul(
                    out=ov[0:65, :],
                    lhsT=cx["vst"][:, c * 65:(c + 1) * 65],
                    rhs=cx["ps"][:, c, qg * 512:(qg + 1) * 512],
                    start=(c == 0), stop=False, skip_group_check=True)
            else:
                j = 4 * qg + (r - 4)
                q0 = (j % 4) * 128
                nc.tensor.matmul(
                    out=ov[0:65, q0:q0 + 128],
                    lhsT=cx["vwt"][:, j * 65:(j + 1) * 65],
                    rhs=cx["pw"][0:112, 128 * j:128 * j + 128],
                    start=False, stop=(r == 7), skip_group_check=True)

        def pv_copy(cx, qg):
            # PSUM -> SBUF staging (DMA cannot read PSUM), all on DVE,
            # fp32 -> fp16 in the copy; out-DMA per half on two queues
            if qg == 0:
                cx["ob"] = p_pool.tile([65, S], dt_in, tag="ob",
                                       name=f"ob{cx['p']}")
            ob = cx["ob"]
            ov = cx["ov" + str(qg)]
            nc.vector.tensor_copy(ob[:, qg * 512:(qg + 1) * 512], ov[0:65, :])
            if qg == 1:
                nc.sync.dma_start(out=out[cx["p"], :, 0:1024],
                                  in_=ob[:, 0:1024])
            if qg == 3:
                nc.gpsimd.dma_start(out=out[cx["p"], :, 1024:2048],
                                    in_=ob[:, 1024:2048])

        ctxs = [load_tiles(0)]
        for pwb in pw_tiles:
            nc.gpsimd.memset(pwb, 0.0)
        for p in range(npairs):
            nxt_needed = p + 1 < npairs
            cur = ctxs[p]
            prev = ctxs[p - 1] if p > 0 else None
            # emit, prefetching next pair's tiles after the first round
            qt, kt, ps, pw = cur["qt"], cur["kt"], cur["ps"], cur["pw"]
            for r in range(8):
                c, g = r % 4, r // 4
                st = s_pool.tile([128, 1024], f32, tag="s")
                for u in range(2):
                    q0 = g * 1024 + u * 512
                    nc.tensor.matmul(
                        out=st[:, u * 512:(u + 1) * 512],
                        lhsT=kt[:, c * 128:(c + 1) * 128],
                        rhs=qt[:, q0:q0 + 512],
                        start=True, stop=True)
                po = ps[:, c, g * 1024:(g + 1) * 1024]
                if c == 3:
                    nc.vector.tensor_scalar(
                        out=po.bitcast(i16), in0=st,
                        scalar1=SCH_A, scalar2=SCH_B, op0=MUL, op1=ADD)
                else:
                    nc.scalar.activation(out=po, in_=st,
                                         func=EXP, scale=SCALE)
                if r == 0 and nxt_needed:
                    ctxs.append(load_tiles(p + 1))
                if prev is not None:
                    for i in range(4 * r, 4 * r + 4):
                        pv_step(prev, i)
                    if r % 2 == 1:
                        pv_copy(prev, r // 2)
            for h in range(2):
                sw = s_pool.tile([128, 1024], f32, tag="s")
                for j in range(8 * h, 8 * h + 8):
                    fo = (j - 8 * h) * 128
                    nc.tensor.matmul(
                        out=sw[0:112, fo:fo + 128],
                        lhsT=cur["ktw"][:, 128 * j:128 * j + 112],
                        rhs=qt[:, 128 * j:128 * j + 128],
                        start=True, stop=True)
                sw4 = sw.rearrange("p (j t f) -> p j t f", t=2, f=64)
                pw4 = pw.rearrange("p (j t f) -> p j t f", t=2, f=64)
                nc.vector.tensor_scalar(
                    out=pw4[0:48, 8 * h:8 * h + 8, 0, :].bitcast(i16),
                    in0=sw4[0:48, 0:8, 0, :],
                    scalar1=SCH_A, scalar2=SCH_B, op0=MUL, op1=ADD)
                nc.vector.tensor_scalar(
                    out=pw4[64:112, 8 * h:8 * h + 8, 1, :].bitcast(i16),
                    in0=sw4[64:112, 0:8, 1, :],
                    scalar1=SCH_A, scalar2=SCH_B, op0=MUL, op1=ADD)
        # flush: PV of the last pair
        last = ctxs[-1]
        for i in range(32):
            pv_step(last, i)
            if i % 8 == 7:
                pv_copy(last, i // 8)

    nc.compile()
    return nc


def _get_nc(dt_in_name="float16"):
    if dt_in_name not in _CACHE:
        _CACHE[dt_in_name] = _build(dt_in_name)
    return _CACHE[dt_in_name]


def _prep_inputs(query, key, value, np_dt):
    q = np.asarray(query).reshape(NPAIRS, S, D)
    k = np.asarray(key).reshape(NPAIRS, S, D)
    v = np.asarray(value).reshape(NPAIRS, S, D)
    kr = k[:, _REORDER, :]
    vr = v[:, _REORDER, :]
    # qkT: [stripe K^T 512 | Q^T 2048 | window K^T padded: pairs of
    # 128 cols as [48 even | 16 zero | 48 odd | 16 zero]]
    KW = 512 + 64 * NW
    qkT = np.zeros((NPAIRS, 64, S + KW), np_dt)
    kTs = kr.transpose(0, 2, 1).astype(np_dt)  # [P, 64, 2048]
    qkT[:, :, 0:512] = kTs[:, :, 0:512]
    qkT[:, :, 512:512 + S] = q.transpose(0, 2, 1)
    kw = kTs[:, :, 512:].reshape(NPAIRS, 64, NW // 2, 2, 48)
    kTw = qkT[:, :, 512 + S:].reshape(NPAIRS, 64, NW // 2, 2, 64)
    kTw[:, :, :, :, 0:48] = kw
    va = np.concatenate(
        [vr, np.ones((NPAIRS, S, 1), vr.dtype)], axis=2).astype(np_dt)
    # vin: stripe V_aug [partition r, chunk c, 65] ++ window-pair V_aug
    # [112 rows (0:48 window 2i, 64:112 window 2i+1, 48:64 zero), i, 65]
    VW = NCH * 65 + (NW // 2) * 65
    vin = np.zeros((NPAIRS, 128, VW), np_dt)
    vin[:, :, 0:NCH * 65] = (
        va[:, :512].reshape(NPAIRS, NCH, 128, 65).transpose(0, 2, 1, 3)
    ).reshape(NPAIRS, 128, NCH * 65)
    vw = va[:, 512:].reshape(NPAIRS, NW // 2, 2, 48, 65)
    vwin = vin[:, :, NCH * 65:].reshape(NPAIRS, 128, NW // 2, 65)
    vwin[:, 0:48] = vw[:, :, 0].transpose(0, 2, 1, 3)
    vwin[:, 64:112] = vw[:, :, 1].transpose(0, 2, 1, 3)
    in_maps = []
    for core in range(NCORES):
        sl = slice(core * P_PER_CORE, (core + 1) * P_PER_CORE)
        in_maps.append({"qkT": np.ascontiguousarray(qkT[sl]),
                        "vin": np.ascontiguousarray(vin[sl])})
    return in_maps


def _run(query, key, value, dt_in_name="float16", trace=False):
    from concourse.bass_utils import run_bass_kernel_spmd
    nc = _get_nc(dt_in_name)
    in_maps = _prep_inputs(query, key, value, np.float16
                           if dt_in_name == "float16" else np.float32)
    res = run_bass_kernel_spmd(nc, in_maps, list(range(NCORES)), trace=trace)
    o = np.concatenate([res.results[i]["out"] for i in range(NCORES)],
                       axis=0).astype(np.float32)
    full = (o[:, 0:64, :] / o[:, 64:65, :]).transpose(0, 2, 1).reshape(
        B, H, S, D).astype(np.float32)
    return full, res


def kernel(query, key, value):
    full, _ = _run(np.asarray(query), np.asarray(key), np.asarray(value))
    return full
